# revision 46
# baseline (speedup 1.0000x reference)
"""Trainium2 Bass kernel for LorentzBatchNorm (bf16-I/O DMA-roofline version).

Math: for points x on the unit hyperboloid (linner(x,x) = -1) and the
normalized centroid `mean` (linner(mean,mean) = -1), the whole module
collapses per point to a rank-1 update:

  alpha = -linner(mean, x)            (one 128-dot per point)
  linner(u,u) = alpha^2 - 1           (u = x - alpha*mean; no 2nd reduction)
  d = arccosh(alpha) = ||x_T||        (Frechet var = mean of d)
  With beta = e0: transport to origin just zeroes channel 0, nu = g*d with
  g = gamma/(var+eps), and

  y[c] = A*x[c] - B*mean[c]  (c >= 1),   y[0] = cosh(nu)
  A = sinh(nu)/sqrt(alpha^2-1)
  B = A * (alpha + u0/(1+mean0)),  u0 = x0 - alpha*mean0

Performance model (TimelineSim): DMA_ENGINES is a single 360 GB/s device.
x and y ship as bf16 (host converts; rel-err gate is 2e-2 and bf16 costs
~2e-3), so per-core traffic is 8.39 MB in + 8.39 MB out = 46.6 us of
transfer = 5.83 us/sample cadence. Each engine's per-sample busy time is
held under that cadence:
  PE   : 32 centroid matmuls + 32 transposes + 32 alpha matmuls + rank-1
         y-producers (K=1 outer products negC1/negB (x) mean into PSUM,
         plus identity@x for ACT-finished tiles)
  ACT  : chain activations + Copy-scale finishes (y = A (*) psum tile)
  DVE  : xbt PSUM->SBUF copies, scalar chain, STT finishes
  Pool : x0/cosh fills, STT finishes
The per-tile finish mode string Y_MODES balances the three vector engines;
XBT_ENGINES balances the transpose-copy work.
"""

import sys

if "/opt/trn_rl_repo" not in sys.path:
    sys.path.insert(0, "/opt/trn_rl_repo")

from contextlib import ExitStack

import numpy as np

import concourse.bass as bass
import concourse.tile as tile
from concourse import mybir
from concourse.vector_clock import ScopedClock

f32 = mybir.dt.float32
bf16 = mybir.dt.bfloat16
ALU = mybir.AluOpType
ACTF = mybir.ActivationFunctionType
X_AXIS = mybir.AxisListType.X

BS, H, W, C = 64, 64, 64, 128
N = H * W  # 4096 points per sample
NCORES = 8
SPB = BS // NCORES  # samples per core
NT = N // 128  # 32 tiles of 128 points
EPS = 1e-5
ACLIP = 1.0 + 1e-7

# --- engine assignment knobs ----------------------------------------------
# xbt transpose-group copies (4 x 8 tiles): 'd'=DVE, 'a'=ACT
XBT_ENGINES = "dddd"
# y pass, per tile (32). Each char picks the finish engine/formula.
# (GPSIMD supports neither PSUM access nor 3-operand STT, so Pool tiles
# split into two 2-operand SBUF ops.)
#  'A' = PE (I@x + K=1 negC1xMrow) -> PSUM; ACT Copy-scale finish (full row)
#  'V' = DVE TSP tmp=Mrow*negB (SBUF bf16); DVE STT finish (cols 1:)
#  'B' = DVE TSP tmp;  Pool TSP y1=x*A;     Pool TT add y=y1+tmp (cols 1:)
Y_MODES = "AAVB" * 5 + "VVAB" * 3
S0_SPLIT = 4  # load chunks for sample 0 (faster pipeline start)
STORE_SPLIT = 8  # tiles per store DMA


# ---------------------------------------------------------------------------
# Tile drain patch: the walrus CoreV3 codegen in this container accepts only
# one sync-wait per CTRL (Drain) instruction, but Tile's final drain piles the
# whole global clock onto a single Drain. Split across chained SP drains.
def _patched_drain_and_barrier(self, tick_clock, wait_clock):
    nc = self.nc
    drain_inst = nc.sync.drain()
    wait_clock.add_sem_waits(
        drain_inst.ins, ScopedClock({None: tick_clock.global_clock})
    )
    si = drain_inst.ins.sync_info
    waits = list(si.on_wait or [])
    if len(waits) > 1:
        si.on_wait = waits[:1]
        for w in waits[1:]:
            d2 = nc.sync.drain()
            si2 = d2.ins.sync_info
            if si2 is None:
                d2.ins.sync_info = mybir.SyncInfo(on_wait=[w], on_update=[])
            else:
                si2.on_wait = [w]
    nc.all_engine_barrier()
    assert self.sems is not None
    popped = nc._tile_sem_poison_stack.pop()
    assert popped is self._sem_poison
    nc.clear_and_free_semaphores(list(self.sems.allocated().values()))
    nc.all_engine_barrier()


_orig_lower_ordered_insts = tile.TileContext._lower_ordered_insts
_wsplit_counter = [0]


def _patched_lower_ordered_insts(self, ordered):
    """Walrus here allows only one sync-wait per instruction; hoist extra
    waits onto same-engine NoOps inserted just before the instruction."""
    maxw = 1
    for insts in ordered.values():
        out = []
        for inst in insts:
            si = inst.sync_info
            waits = list(si.on_wait) if si is not None and si.on_wait else []
            if len(waits) > maxw:
                extra, keep = waits[:-maxw], waits[-maxw:]
                for i in range(0, len(extra), maxw):
                    _wsplit_counter[0] += 1
                    nop = mybir.InstNoOp(
                        name=f"wsplit-{_wsplit_counter[0]}",
                        engine=inst.engine,
                        ins=[],
                        outs=[],
                        sync_info=mybir.SyncInfo(
                            on_wait=extra[i : i + maxw], on_update=[]
                        ),
                    )
                    out.append(nop)
                si.on_wait = keep
            out.append(inst)
        insts[:] = out
    return _orig_lower_ordered_insts(self, ordered)


def _install_tile_patch():
    tile.TileContext._drain_and_barrier = _patched_drain_and_barrier
    tile.TileContext._lower_ordered_insts = _patched_lower_ordered_insts


# ---------------------------------------------------------------------------


class _Ctx:
    """Shared build-time state (pools, constants, per-sample tiles)."""


def _emit_cent_chunk(nc, cx, s, i):
    """Sample s: 8 centroid matmuls of chunk i (bf16 x read directly)."""
    x_sb = cx.x_sbs[s]
    if i == 0:
        cx.psSs[s] = cx.psS.tile([128, C], f32, tag="psS", name="psS")
    psS = cx.psSs[s]
    for k in range(8):
        t = 8 * i + k
        nc.tensor.matmul(
            psS, cx.ones, x_sb[:, t, :], start=(t == 0), stop=(t == NT - 1)
        )


def _emit_stats(nc, cx, s):
    """Sample s: centroid stats -> Mrow_bf, mean0, i1p, Wbcol."""
    # ---- stats
    S_sb = cx.rows.tile([128, C], f32, tag="S")
    nc.vector.tensor_copy(S_sb, cx.psSs[s])

    scr = cx.rows.tile([128, C], f32, tag="scr")
    ss = cx.chain.tile([128, 1], f32, tag="ss")
    nc.vector.scalar_tensor_tensor(
        out=scr, in0=S_sb, scalar=1.0, in1=S_sb,
        op0=ALU.bypass, op1=ALU.mult, accum_out=ss,
    )
    s0sq = cx.chain.tile([128, 1], f32, tag="s0sq")
    nc.vector.tensor_scalar(
        out=s0sq, in0=S_sb[:, 0:1], scalar1=S_sb[:, 0:1], scalar2=None,
        op0=ALU.mult,
    )
    nls = cx.chain.tile([128, 1], f32, tag="nls")  # -linner(S,S) = 2*S0^2 - ss
    nc.vector.tensor_scalar(
        out=nls, in0=s0sq, scalar1=2.0, scalar2=ss, op0=ALU.mult,
        op1=ALU.subtract,
    )
    h1 = cx.chain.tile([128, 1], f32, tag="h1")  # sqrt(-linner(S,S))
    nc.scalar.activation(h1, nls, ACTF.Sqrt)
    rn = cx.chain.tile([128, 1], f32, tag="rn")  # 1/sqrt(...)
    nc.vector.reciprocal(rn, h1)
    Mrow_bf = cx.stats.tile([128, C], bf16, tag="Mrow_bf")
    nc.vector.tensor_scalar_mul(Mrow_bf, S_sb, rn)
    mean0 = cx.stats.tile([128, 1], f32, tag="mean0")
    nc.gpsimd.tensor_mul(mean0, S_sb[:, 0:1], rn)
    t1p = cx.chain.tile([128, 1], f32, tag="t1p")
    nc.gpsimd.tensor_scalar_add(t1p, mean0, 1.0)
    i1p = cx.stats.tile([128, 1], f32, tag="i1p")  # 1/(1+mean0)
    nc.vector.reciprocal(i1p, t1p)
    # W column for alpha matmuls: w = -mean_c except w[0] = +mean0. Obtained
    # by transposing the (all-rows-equal) bf16 mean row, then a sign flip.
    # (Rides in the pt tag's bank ring to stay within the 8 PSUM banks.)
    pst = cx.psT.tile([128, 8, C], bf16, tag="pt")
    nc.tensor.transpose(pst[:, 0, :], Mrow_bf, cx.ident_b)
    Wbcol = cx.stats.tile([128, 1], bf16, tag="Wbcol")
    nc.vector.tensor_mul(Wbcol, pst[:, 0, 0:1], cx.signc)
    cx.bulk[s] = {
        "Mrow_bf": Mrow_bf, "mean0": mean0, "i1p": i1p, "Wbcol": Wbcol,
    }


def _emit_alpha_group(nc, cx, s, grp):
    """Sample s group grp: 8 PE transposes of bf16 x -> PSUM, one group copy
    to SBUF, 8 [128ch,128pt]x[128ch,1] matmuls -> alpha columns in pa."""
    x_sb = cx.x_sbs[s]
    eng = {"d": nc.vector, "a": nc.scalar, "p": nc.gpsimd}
    if grp == 0:
        # chv packs the alpha columns [0:NT] and the pv scalar [NT]
        chv = cx.small.tile([128, NT + 1], f32, tag="chv", name="chv")
        cx.pas[s] = chv[:, 0:NT]
        cx.pvs[s] = chv[:, NT : NT + 1]
        cx.bulk[s]["pa"] = cx.pas[s]
    pa = cx.bulk[s]["pa"]
    pt = cx.psT.tile([128, 8, C], bf16, tag="pt")
    for k in range(8):
        nc.tensor.transpose(pt[:, k, :], x_sb[:, 8 * grp + k, :], cx.ident_b)
    xbt = cx.rows.tile([128, 8, C], bf16, tag="xbt")
    e = XBT_ENGINES[grp]
    if e == "a":
        nc.scalar.copy(
            xbt.rearrange("p a c -> p (a c)"), pt.rearrange("p a c -> p (a c)")
        )
    else:
        eng[e].tensor_copy(
            xbt.rearrange("p a c -> p (a c)"), pt.rearrange("p a c -> p (a c)")
        )
    Wbcol = cx.bulk[s]["Wbcol"]
    for k in range(8):
        t = 8 * grp + k
        nc.tensor.matmul(
            pa[:, t : t + 1], xbt[:, k, :], Wbcol, start=True, stop=True
        )


def _emit_chain_phase(nc, cx, s, phase):
    """Sample s per-point scalar chain [128, NT], split into 3 phases so
    that each engine has at most ~2 stalled ops per phase (the 4-deep
    wait queues can then bypass into following ready work)."""
    b = cx.bulk[s]
    if phase == 1:
        al = cx.chain.tile([128, NT], f32, tag="al")
        nc.vector.tensor_scalar_max(al, b["pa"], ACLIP)
        asq = cx.chain.tile([128, NT], f32, tag="asq")
        nc.vector.tensor_mul(asq, al, al)
        r2 = cx.chain.tile([128, NT], f32, tag="r2")  # 2*sqrt(alpha^2-1)
        nc.scalar.activation(r2, asq, ACTF.Sqrt, scale=4.0, bias=cx.bm4)
        rinv05 = cx.chain.tile([128, NT], f32, tag="rinv05")  # 1/(2r)
        nc.vector.reciprocal(rinv05, r2)
        z = cx.chain.tile([128, NT], f32, tag="z")  # alpha + r = e^d
        nc.vector.scalar_tensor_tensor(
            out=z, in0=r2, scalar=0.5, in1=al, op0=ALU.mult, op1=ALU.add
        )
        b.update(al=al, rinv05=rinv05, z=z)
    elif phase == 2:
        al, z = b["al"], b["z"]
        # d = arccosh(alpha); the activation's accumulator gives sum(d) free
        d = cx.chain.tile([128, NT], f32, tag="d")
        dsum = cx.chain.tile([128, 1], f32, tag="dsum")
        nc.scalar.activation(d, z, ACTF.Ln, accum_out=dsum)
        x_sb = cx.x_sbs[s]
        x0 = x_sb[:, :, 0:1].rearrange("p t c -> p (t c)")
        negu0 = cx.chain.tile([128, NT], f32, tag="negu0")  # alpha*mean0 - x0
        nc.vector.scalar_tensor_tensor(
            out=negu0, in0=al, scalar=b["mean0"], in1=x0,
            op0=ALU.mult, op1=ALU.subtract,
        )
        # nb2 holds negC1_bf for the PE transpose to rank-1 producer rows
        nb2 = cx.stats.tile([128, NT], bf16, tag="nb2")
        nc.vector.scalar_tensor_tensor(
            out=nb2, in0=negu0, scalar=b["i1p"], in1=al,
            op0=ALU.mult, op1=ALU.subtract,
        )
        if False and s == 0:
            # warmup samples bypass the nbTf flatten (it would queue behind
            # the bulk loads on the DMA device): their A-tile producers run
            # on DVE from a f32 negC1
            negC1f = cx.stats.tile([128, NT], f32, tag="negC1f")
            nc.vector.scalar_tensor_tensor(
                out=negC1f, in0=negu0, scalar=b["i1p"], in1=al,
                op0=ALU.mult, op1=ALU.subtract,
            )
            b.update(negC1f=negC1f)
        # transpose + flatten the negC1 half now: the A-tile producers can
        # then start early next period without waiting on chain phase 3
        pnb = cx.small.tile([NT, 128], bf16, tag="pnb")
        nbT = cx.stats.tile([NT, 128], bf16, tag="nbT")
        nbTf = cx.stats.tile([1, NT * 128], bf16, tag="nbTf")
        nc.tensor.transpose(pnb, nb2, cx.ident_b)
        nc.vector.tensor_copy(nbT, pnb)
        nc.sync.dma_start(out=nbTf, in_=nbT)
        b.update(pnb=pnb, nbT=nbT, nbTf=nbTf)
        # var = mean(d) (eps << var is dropped); g = gamma*N / sum(d)
        pv = cx.pvs[s]
        nc.tensor.matmul(pv, cx.ones_f, dsum, start=True, stop=True)
        rv = cx.chain.tile([128, 1], f32, tag="rv")
        nc.vector.reciprocal(rv, pv)
        g = cx.chain.tile([128, 1], f32, tag="g")
        nc.vector.tensor_scalar(
            out=g, in0=rv, scalar1=cx.gamma_col, scalar2=float(N),
            op0=ALU.mult, op1=ALU.mult,
        )
        b.update(d=d, nb2=nb2, g=g)
    else:
        d, g, rinv05 = b["d"], b["g"], b["rinv05"]
        nu = cx.chain.tile([128, NT], f32, tag="nu")
        nc.vector.tensor_scalar_mul(nu, d, g)
        E2 = cx.chain.tile([128, NT], f32, tag="E2")  # e^nu
        nc.scalar.activation(E2, nu, ACTF.Exp)
        Ei2 = cx.chain.tile([128, NT], f32, tag="Ei2")  # e^-nu
        nc.vector.reciprocal(Ei2, E2)
        sh2 = cx.chain.tile([128, NT], f32, tag="sh2")  # 2*sinh(nu)
        nc.vector.tensor_sub(sh2, E2, Ei2)
        ch2 = cx.stats.tile([128, NT], f32, tag="ch2")  # 2*cosh(nu)
        nc.gpsimd.tensor_add(ch2, E2, Ei2)
        A = cx.stats.tile([128, NT], f32, tag="A")  # sinh(nu)/r
        nc.vector.tensor_mul(A, sh2, rinv05)
        negB = cx.stats.tile([128, NT], f32, tag="negB")  # -B = A*negC1
        nc.vector.tensor_mul(negB, A, b["nb2"])
        cx.front[s] = (b["Mrow_bf"], negB, A, ch2, b["nbTf"])
        cx.warm_negC1f[s] = b.get("negC1f")


def _emit_y_producers(nc, cx, s, hc):
    """Sample s half-chunk hc (4 tiles): per-tile rank-1 PE producers (run a
    half-chunk ahead of the finishes so consumer engines never stall).
    PSUM producer tiles are [128, 4, C] = exactly one 2KB bank."""
    x_sb = cx.x_sbs[s]
    Mrow_bf, negB, A, ch2, nbTf = cx.front[s]
    mrow = Mrow_bf[0:1, :]
    nbr = nbTf.rearrange("p (t b) -> p t b", b=128)
    modes = {Y_MODES[4 * hc + j] for j in range(4)}
    psV = None
    if modes & {"A", "D"} and cx.warm_negC1f.get(s) is None:
        psV = cx.psV.tile([128, 4, C], f32, tag="psV")
    cx.psVs[(s, hc)] = psV
    negC1f = cx.warm_negC1f.get(s)
    for j in range(4):
        t = 4 * hc + j
        mode = Y_MODES[t]
        if mode == "A" and negC1f is not None:
            tmp = cx.amnegp.tile([128, C], bf16, tag="amneg")
            nc.vector.scalar_tensor_tensor(
                out=tmp, in0=Mrow_bf, scalar=negC1f[:, t : t + 1],
                in1=x_sb[:, t, :], op0=ALU.mult, op1=ALU.add,
            )
            cx.ytmp[(s, t)] = tmp
        elif mode == "A":
            # psV_j = x_t + negC1_t (x) mean   (finished by ACT scale-copy)
            nc.tensor.matmul(
                psV[:, j, :], cx.ident_b, x_sb[:, t, :], start=True, stop=False
            )
            nc.tensor.matmul(
                psV[:, j, :], nbr[0:1, t, :], mrow, start=False, stop=True
            )
        else:
            # V/B: SBUF bf16 tmp = Mrow * negB_t
            tmp = cx.amnegp.tile([128, C], bf16, tag="amneg")
            nc.vector.tensor_scalar_mul(tmp, Mrow_bf, negB[:, t : t + 1])
            cx.ytmp[(s, t)] = tmp
            if mode == "B":
                y1 = cx.amnegp.tile([128, C], bf16, tag="y1")
                nc.gpsimd.tensor_scalar_mul(y1, x_sb[:, t, :], A[:, t : t + 1])
                cx.ytmp[(s, t, "y1")] = y1


def _emit_y_finishes(nc, cx, s, hc):
    """Sample s half-chunk hc: y finishes in place over x_sb (cols 1:),
    col 0 <- cosh, store every STORE_SPLIT tiles."""
    x_sb = cx.x_sbs[s]
    Mrow_bf, negB, A, ch2, nbTf = cx.front[s]
    psV = cx.psVs.pop((s, hc))
    ys = cx.y_d[s * N : (s + 1) * N, :].rearrange("(p t) c -> p t c", t=NT)
    for j in range(4):
        t = 4 * hc + j
        mode = Y_MODES[t]
        if mode == "A" and (s, t) in cx.ytmp:
            tmp = cx.ytmp.pop((s, t))
            nc.scalar.activation(
                x_sb[:, t, :], tmp, ACTF.Copy, scale=A[:, t : t + 1]
            )
        elif mode == "A":
            nc.scalar.activation(
                x_sb[:, t, :], psV[:, j, :], ACTF.Copy, scale=A[:, t : t + 1]
            )
        elif mode == "V":
            tmp = cx.ytmp.pop((s, t))
            nc.vector.scalar_tensor_tensor(
                out=x_sb[:, t, 1:C], in0=x_sb[:, t, 1:C],
                scalar=A[:, t : t + 1], in1=tmp[:, 1:C],
                op0=ALU.mult, op1=ALU.add,
            )
        else:
            tmp = cx.ytmp.pop((s, t))
            y1 = cx.ytmp.pop((s, t, "y1"))
            nc.gpsimd.tensor_tensor(
                x_sb[:, t, 1:C], y1[:, 1:C], tmp[:, 1:C], ALU.add
            )
    # col 0 <- cosh = ch2/2 once per full chunk, then store
    lo, hi = 4 * hc, 4 * (hc + 1)
    if hi % 8 == 0:
        nc.gpsimd.tensor_scalar(
            out=x_sb[:, hi - 8 : hi, 0:1].rearrange("p t c -> p (t c)"),
            in0=ch2[:, hi - 8 : hi],
            scalar1=0.5, scalar2=None, op0=ALU.mult,
        )
    if hi % STORE_SPLIT == 0:
        slo = hi - STORE_SPLIT
        nc.sync.dma_start(out=ys[:, slo:hi, :], in_=x_sb[:, slo:hi, :])


def build_program():
    _install_tile_patch()
    nc = bass.Bass("TRN2", debug=False)
    x_d = nc.dram_tensor("x", [SPB * N, C], bf16, kind="ExternalInput").ap()
    g_d = nc.dram_tensor("gamma", [1], f32, kind="ExternalInput").ap()
    i_d = nc.dram_tensor("ident", [128, 128], bf16, kind="ExternalInput").ap()
    y_d = nc.dram_tensor("y", [SPB * N, C], bf16, kind="ExternalOutput").ap()

    with tile.TileContext(nc) as tc, ExitStack() as ctx:
        cx = _Ctx()
        cx.y_d = y_d
        singles = ctx.enter_context(tc.tile_pool(name="singles", bufs=1))
        cx.xpool = ctx.enter_context(tc.tile_pool(name="x", bufs=SPB))
        cx.rows = ctx.enter_context(tc.tile_pool(name="rows", bufs=4))
        cx.amnegp = ctx.enter_context(tc.tile_pool(name="amneg", bufs=12))
        cx.chain = ctx.enter_context(tc.tile_pool(name="chain", bufs=4))
        cx.stats = ctx.enter_context(tc.tile_pool(name="stats", bufs=4))
        cx.psS = ctx.enter_context(tc.tile_pool(name="psS", bufs=2, space="PSUM"))
        cx.psT = ctx.enter_context(tc.tile_pool(name="psT", bufs=2, space="PSUM"))
        cx.psV = ctx.enter_context(tc.tile_pool(name="psV", bufs=2, space="PSUM"))
        cx.small = ctx.enter_context(
            tc.tile_pool(name="small", bufs=1, space="PSUM")
        )

        # first x chunk goes first so the DMA pipe fills immediately; the
        # tiny constant loads then ride behind it
        xs0 = x_d[0:N, :].rearrange("(p t) c -> p t c", t=NT)
        x_sb0 = cx.xpool.tile([128, NT, C], bf16, tag="xsb", name="xsb0")
        step0 = NT // S0_SPLIT
        nc.sync.dma_start(out=x_sb0[:, 0:step0, :], in_=xs0[:, 0:step0, :])
        cx.ones = singles.tile([128, 128], bf16)
        nc.vector.memset(cx.ones, 1.0)
        cx.ones_f = singles.tile([128, 128], f32)
        nc.vector.memset(cx.ones_f, 1.0)
        cx.bm4 = singles.tile([128, 1], f32)
        nc.vector.memset(cx.bm4, -4.0)

        cx.signc = singles.tile([128, 1], f32)
        nc.vector.memset(cx.signc, -1.0)
        nc.vector.memset(cx.signc[0:1, 0:1], 1.0)

        # s0 loads (quartered for a fast pipeline start) + constants + s1;
        # later samples load just-in-time (one per period) so the DMA device
        # never backlogs ahead of the latency-critical flatten DMAs/stores
        cx.x_sbs = [x_sb0]
        for i in range(1, S0_SPLIT):
            nc.sync.dma_start(
                out=x_sb0[:, i * step0 : (i + 1) * step0, :],
                in_=xs0[:, i * step0 : (i + 1) * step0, :],
            )
        cx.ident_b = singles.tile([128, 128], bf16)
        nc.sync.dma_start(out=cx.ident_b, in_=i_d)
        cx.gamma_col = singles.tile([128, 1], f32)
        nc.sync.dma_start(out=cx.gamma_col, in_=g_d.to_broadcast((128, 1)))

        def emit_load(s, eng=None):
            if s >= SPB or len(cx.x_sbs) > s:
                return
            xs = x_d[s * N : (s + 1) * N, :].rearrange("(p t) c -> p t c", t=NT)
            x_sb = cx.xpool.tile([128, NT, C], bf16, tag="xsb")
            (eng or nc.sync).dma_start(out=x_sb, in_=xs)
            cx.x_sbs.append(x_sb)

        for s in range(1, SPB):
            emit_load(s)

        # Fine-grained interleaved emission. Period p overlaps three samples:
        # y chunks of p-1 (producers one chunk ahead of finishes), chain
        # phases of p, centroid chunks + bulk of p+1.
        cx.bulk = {}
        cx.front = {}
        cx.psSs = {}
        cx.psVs = {}
        cx.ytmp = {}
        cx.warm_negC1f = {}
        cx.pas = {}
        cx.pvs = {}

        # warmup: sample 0 front entirely
        for i in range(4):
            _emit_cent_chunk(nc, cx, 0, i)
        _emit_stats(nc, cx, 0)
        for grp in range(4):
            _emit_alpha_group(nc, cx, 0, grp)
        for p in range(SPB + 1):
            # Period p interleaves three samples: the y pass of p-1 (producers
            # one half-chunk ahead of finishes) is spread evenly through the
            # period against chain(p) and cent+stats+alpha(p+1), so every
            # engine's in-order queue always has ready work near its head.
            have_y = 1 <= p <= SPB
            have_ch = p < SPB
            have_next = p + 1 < SPB

            def y_prod(h):
                if have_y and h < 8:
                    _emit_y_producers(nc, cx, p - 1, h)

            def y_fin(h):
                if have_y:
                    _emit_y_finishes(nc, cx, p - 1, h)

            if have_next:
                _emit_cent_chunk(nc, cx, p + 1, 0)
            y_prod(0)
            y_prod(1)
            if have_ch:
                _emit_chain_phase(nc, cx, p, 1)
            if have_next:
                _emit_cent_chunk(nc, cx, p + 1, 1)
            y_fin(0)
            y_prod(2)
            if have_next:
                _emit_cent_chunk(nc, cx, p + 1, 2)
            y_fin(1)
            y_prod(3)
            if have_ch:
                _emit_chain_phase(nc, cx, p, 2)
            if have_next:
                _emit_cent_chunk(nc, cx, p + 1, 3)
            y_fin(2)
            y_prod(4)
            if have_next:
                _emit_stats(nc, cx, p + 1)
            y_fin(3)
            y_prod(5)
            if have_next:
                _emit_alpha_group(nc, cx, p + 1, 0)
            y_fin(4)
            y_prod(6)
            if have_ch:
                _emit_chain_phase(nc, cx, p, 3)
            if have_next:
                _emit_alpha_group(nc, cx, p + 1, 1)
            y_fin(5)
            y_prod(7)
            if have_next:
                _emit_alpha_group(nc, cx, p + 1, 2)
            y_fin(6)
            if have_next:
                _emit_alpha_group(nc, cx, p + 1, 3)
            y_fin(7)
    return nc


_PROGRAM = None


def _get_program():
    global _PROGRAM
    if _PROGRAM is None:
        _PROGRAM = build_program()
    return _PROGRAM


def _numpy_reference(x, beta, gamma):
    """Full-precision numpy fallback (general beta)."""
    CLAMP = 1e-8
    bs, h, w, c = x.shape
    x = x.reshape(bs, h * w, c).astype(np.float64)
    beta = beta.astype(np.float64)
    e0 = np.zeros(c)
    e0[0] = 1.0

    def linner(a, b):
        return (a * b).sum(-1, keepdims=True) - 2.0 * a[..., :1] * b[..., :1]

    m = x.mean(1, keepdims=True)
    mean = m / np.sqrt(np.clip(-linner(m, m), CLAMP, None))
    alpha = np.clip(-linner(mean, x), 1.0 + 1e-7, None)
    u = x - alpha * mean
    un = np.sqrt(np.clip(linner(u, u), CLAMP, None))
    x_T = np.arccosh(alpha) * u / un
    x_T = x_T - (x_T[..., :1] / (1.0 + mean[..., :1])) * (mean + e0)
    var = np.linalg.norm(x_T, axis=-1).mean(1)[:, None, None]
    x_T = x_T * (gamma.astype(np.float64) / (var + EPS))
    x_T = x_T + (linner(beta, x_T) / (1.0 + beta[0])) * (beta + e0)
    nu = np.sqrt(np.clip(linner(x_T, x_T), CLAMP, None))
    out = np.cosh(nu) * beta + np.sinh(nu) * x_T / nu
    return out.reshape(bs, h, w, c).astype(np.float32)


def kernel(x, beta, gamma):
    x = np.ascontiguousarray(x, dtype=np.float32)
    beta = np.asarray(beta, dtype=np.float32)
    gamma = np.asarray(gamma, dtype=np.float32).reshape(1)

    e0 = np.zeros(C, np.float32)
    e0[0] = 1.0
    if not np.array_equal(beta, e0):
        return _numpy_reference(x, beta, gamma)

    from concourse.bass_utils import run_bass_kernel_spmd

    import ml_dtypes

    nc = _get_program()
    xr = x.reshape(BS * N, C).astype(ml_dtypes.bfloat16)
    ident = np.eye(128, dtype=ml_dtypes.bfloat16)
    in_maps = [
        {"x": xr[s * SPB * N : (s + 1) * SPB * N], "gamma": gamma, "ident": ident}
        for s in range(NCORES)
    ]
    res = run_bass_kernel_spmd(nc, in_maps, core_ids=list(range(NCORES)))
    y = np.concatenate(
        [np.asarray(r["y"], dtype=np.float32) for r in res.results], axis=0
    )
    return y.reshape(BS, H, W, C)
